# revision 28
# baseline (speedup 1.0000x reference)
"""TopK sparse autoencoder (encode -> per-token top-100 mask -> decode) on 8 TRN2 cores.

Sharding: data-parallel over the 4096-token batch (512 tokens/core), weights
replicated. Per core:
  pre  = (x - b_dec) @ W_enc + b_enc       (single-pass float32r matmul on PE)
  t    = 100th largest of relu(pre) per token (DVE max8/match_replace:
         top-16 of each 512-wide chunk extracted from the evacuated copy,
         then exact top-100 of the candidate set)
  E    = pre * (pre >= t)                  (masked in transposed layout)
  xhat = E @ W_dec + b_dec                 (bf16 matmul, E^T tiles stationary)

float32r (e8m11) runs the PE at 1 cycle/row (same as bf16) while keeping 12
significand bits: pre error ~2e-4, which keeps top-100 selection swaps rare
enough for ~1.5e-2 end-to-end relative error. Operands are pre-rounded on the
host (RNE to 11 mantissa bits) as the BIR verifier requires, and x is also
pre-transposed on the host so encode needs no PE transposes.

Structure per core:
  phase 1: ONE W_enc sweep; all 4 token tiles accumulate in 4 PSUM banks per
           512-wide chunk. PSUM evacuates via ACT into one combined SBUF tile,
           spills with one DMA per chunk (pre is 16MB/core), and top-16
           candidates per chunk are extracted destructively from the copy.
           After chunk 24 the candidate buffers compress to top-104 during
           DVE slack, shortening the inter-phase threshold chains.
  phase 2: short threshold chains; stream pre back (one DMA per chunk),
           DVE mask -> bf16, bf16 PE transposes into a quad E^T
           (128KB/partition, phase-scoped pools make room). Decode quarter 0
           chases mask production; ONE W_dec sweep total, DIN quarter at a
           time (4 tiles x 4 PSUM banks).

The top-16-per-chunk candidate set contains the global top-100 as long as no
512-chunk holds more than 16 of a row's top-100 (iid inputs: max observed 15
over 4096 rows; a single miss costs ~one swapped atom on one token).
"""
import numpy as np
import ml_dtypes

import concourse.bacc as bacc
import concourse.mybir as mybir
from concourse.tile import TileContext
from concourse.bass_utils import run_bass_kernel_spmd

B, DIN, DSAE, TOPK = 4096, 2048, 16384, 100
NCORES = 8
TPC = B // NCORES            # 512 tokens per core
MT = TPC // 128              # 4 token tiles per core
CH = 512                     # encode chunk width == one PSUM bank (fp32)
NCH = DSAE // CH             # 32 chunks
KTE = DIN // 128             # 16 contraction slices for encode
KTD = DSAE // 128            # 128 contraction slices for decode
R_EXT = 2                    # extraction rounds per chunk -> top-16 candidates
KG = 8                       # k-slices fetched per W_enc DMA
KD = 4                       # k-slices fetched per W_dec DMA
NQ = 4                       # decode DIN quarters (PSUM: 4 tiles x 512 fp32)
CCUT = 24                    # candidate compression point (chunks 0..CCUT-1)
NC1 = CCUT * R_EXT * 8       # 384 candidates before compression
NROUND = (TOPK + 7) // 8     # 13 max8 rounds reach rank 100
NC2 = NROUND * 8 + (NCH - CCUT) * R_EXT * 8   # 104 + 128 compressed set
NEG = -1e30

_cache = {}


def round_fp32r(a):
    """RNE to 11 mantissa bits (e8m11) stored as fp32 — the PE's fp32r format."""
    a = np.ascontiguousarray(a, dtype=np.float32)
    u = a.view(np.uint32).astype(np.uint64)
    low = u & 0xFFF
    u = u & ~np.uint64(0xFFF)
    lsb = (u >> 12) & 1
    roundup = (low > 0x800) | ((low == 0x800) & (lsb == 1))
    u = u + np.where(roundup, np.uint64(0x1000), np.uint64(0))
    return (u & 0xFFFFFFFF).astype(np.uint32).view(np.float32)


def _build(with_benc: bool, with_bdec: bool):
    key = (with_benc, with_bdec)
    if key in _cache:
        return _cache[key]

    nc = bacc.Bacc()
    xT_d = nc.dram_tensor("xT", [DIN, TPC], mybir.dt.float32r, kind="ExternalInput")
    we_d = nc.dram_tensor("w_enc", [DIN, DSAE], mybir.dt.float32r, kind="ExternalInput")
    be_d = nc.dram_tensor("b_enc", [1, DSAE], mybir.dt.float32, kind="ExternalInput")
    wd_d = nc.dram_tensor("w_dec", [DSAE, DIN], mybir.dt.bfloat16, kind="ExternalInput")
    bd_d = nc.dram_tensor("b_dec", [1, DIN], mybir.dt.float32, kind="ExternalInput")
    out_d = nc.dram_tensor("xhat", [TPC, DIN], mybir.dt.float32, kind="ExternalOutput")
    # spill layout: [128, chunk, tile, 512]
    pre_d = nc.dram_tensor("pre_scratch", [128, NCH * MT * CH], mybir.dt.float32)

    with TileContext(nc) as tc:
        with tc.tile_pool(name="cst", bufs=1) as cst, \
             tc.tile_pool(name="st", bufs=2) as st:

            be_sb = bd_bc = ones1 = None
            if with_benc:
                be_sb = cst.tile([1, DSAE], mybir.dt.float32, tag="be")
                nc.sync.dma_start(be_sb, be_d[:, :])
                ones1 = cst.tile([1, 128], mybir.dt.float32, tag="ones")
                nc.vector.memset(ones1, 1.0)
            if with_bdec:
                bd_row = cst.tile([1, DIN], mybir.dt.float32, tag="bdr")
                nc.sync.dma_start(bd_row, bd_d[:, :])
                bd_bc = cst.tile([128, DIN], mybir.dt.float32, tag="bdb")
                nc.gpsimd.partition_broadcast(bd_bc, bd_row)

            cands = [st.tile([128, NC1], mybir.dt.float32, tag="cands", bufs=MT,
                             name=f"cands_{m}") for m in range(MT)]
            fcand = [st.tile([128, NC2], mybir.dt.float32, tag="fcand", bufs=MT,
                             name=f"fcand_{m}") for m in range(MT)]

            def extract(src, c, m):
                """Destructive top-(8*R_EXT) extraction from an SBUF chunk."""
                for r in range(R_EXT):
                    if c < CCUT:
                        off = (c * R_EXT + r) * 8
                        m8 = cands[m][:, off:off + 8]
                    else:
                        off = NROUND * 8 + ((c - CCUT) * R_EXT + r) * 8
                        m8 = fcand[m][:, off:off + 8]
                    nc.vector.max(out=m8, in_=src)
                    if r < R_EXT - 1:
                        nc.vector.match_replace(out=src, in_to_replace=m8,
                                                in_values=src, imm_value=NEG)

            def compress(m, r0, r1):
                """cands[m] (384 wide) -> top-104 into fcand[m][:, :104]."""
                for r in range(r0, r1):
                    m8 = fcand[m][:, r * 8:(r + 1) * 8]
                    nc.vector.max(out=m8, in_=cands[m])
                    if r < NROUND - 1:
                        nc.vector.match_replace(out=cands[m], in_to_replace=m8,
                                                in_values=cands[m], imm_value=NEG)

            def threshold_of(m):
                """fcand[m] -> rank-100 threshold column [128, 1] (clamped > 0)."""
                s8 = st.tile([128, 8], mybir.dt.float32, tag="s8", name=f"s8_{m}")
                for r in range(NROUND):
                    nc.vector.max(out=s8, in_=fcand[m])
                    if r < NROUND - 1:
                        nc.vector.match_replace(out=fcand[m], in_to_replace=s8,
                                                in_values=fcand[m], imm_value=NEG)
                t_col = st.tile([128, 1], mybir.dt.float32, tag="tcol", bufs=4,
                                name=f"tc_{m}")
                nc.vector.tensor_scalar_max(
                    t_col, s8[:, (TOPK - 1) % 8:(TOPK - 1) % 8 + 1], 1e-30)
                return t_col

            # ---- phase 1: encode all 4 tiles in ONE W_enc sweep ----
            with tc.tile_pool(name="enc", bufs=2) as enc_p, \
                 tc.tile_pool(name="ps1", bufs=8, space="PSUM") as psp1:
                # host-pre-transposed x: [din, tok] fp32r, all 4 token tiles
                xT = enc_p.tile([128, KTE * TPC], mybir.dt.float32r, tag="xT",
                                bufs=1)
                xT3 = xT.rearrange("p (k t) -> p k t", k=KTE)
                xTd3 = xT_d.rearrange("(k p) t -> p k t", p=128)
                nc.sync.dma_start(xT3[:, 0:1, :], xTd3[:, 0:1, :])
                nc.gpsimd.dma_start(xT3[:, 1:4, :], xTd3[:, 1:4, :])
                nc.gpsimd.dma_start(xT3[:, 4:KTE, :], xTd3[:, 4:KTE, :])

                for c in range(NCH):
                    csl = slice(c * CH, (c + 1) * CH)
                    ps = [psp1.tile([128, CH], mybir.dt.float32, tag="ps",
                                    name=f"pse_{c}_{m}") for m in range(MT)]
                    kgrp = 4 if c < 2 else KG   # small fetches to fill pipe
                    for kg in range(KTE // kgrp):
                        rows = slice(kg * kgrp * 128, (kg + 1) * kgrp * 128)
                        wt = enc_p.tile([128, KG * CH], mybir.dt.float32r,
                                        tag="we", bufs=2, name=f"wt_{c}_{kg}")
                        wts = wt[:, :kgrp * CH]
                        nc.sync.dma_start(
                            wts.rearrange("p (k n) -> p k n", k=kgrp),
                            we_d[rows, csl].rearrange("(k p) n -> p k n", p=128))
                        wt3 = wts.rearrange("p (k n) -> p k n", k=kgrp)
                        for kk in range(kgrp):
                            k = kg * KG + kk
                            last = (k == KTE - 1) and not with_benc
                            for m in range(MT):
                                nc.tensor.matmul(ps[m],
                                                 xT3[:, k, m * 128:(m + 1) * 128],
                                                 wt3[:, kk, :],
                                                 start=(k == 0), stop=last)
                    sc = enc_p.tile([128, MT * CH], mybir.dt.float32, tag="sc",
                                    bufs=6, name=f"sc_{c}")
                    for m in range(MT):
                        if with_benc:
                            nc.tensor.matmul(ps[m], ones1, be_sb[:, csl],
                                             start=False, stop=True)
                        # ACT evacuates; PSUM bank frees at copy end
                        nc.scalar.copy(sc[:, m * CH:(m + 1) * CH], ps[m])
                    # Pool-queue DMA: keeps the sync queue a pure W_enc
                    # prefetch stream (no head-of-line blocking)
                    nc.gpsimd.dma_start(
                        pre_d[:, c * MT * CH:(c + 1) * MT * CH], sc)
                    for m in range(MT):
                        extract(sc[:, m * CH:(m + 1) * CH], c, m)
                    if c >= CCUT:
                        m2 = (c - CCUT) // 2
                        r0, r1 = ((0, 7) if (c - CCUT) % 2 == 0
                                  else (7, NROUND))
                        compress(m2, r0, r1)

            tcols = []

            # ---- phase 2: stream pre back, mask -> quad E^T, decode ----
            with tc.tile_pool(name="dec", bufs=2) as dec_p, \
                 tc.tile_pool(name="psd", bufs=8, space="PSUM") as psp3:
                # E^T for all 4 token tiles: column = k*512 + m*128 + tok
                eT = dec_p.tile([128, KTD * TPC], mybir.dt.bfloat16, tag="eT",
                                bufs=1)
                eT3 = eT.rearrange("p (k t) -> p k t", t=TPC)

                def mask_chunk(q):
                    """Stream pre chunk q of all 4 tiles, mask, T -> eT."""
                    mrl = st.tile([128, MT * CH], mybir.dt.float32, tag="mrl",
                                  bufs=3, name=f"mrl_{q}")
                    nc.sync.dma_start(mrl, pre_d[:, q * MT * CH:(q + 1) * MT * CH])
                    for m in range(MT):
                        # mask in fp32 math, write bf16 (bf16 transposes 2x)
                        mrb = st.tile([128, CH], mybir.dt.bfloat16, tag="mrb",
                                      bufs=3, name=f"mrb_{q}_{m}")
                        nc.vector.scalar_tensor_tensor(
                            out=mrb, in0=mrl[:, m * CH:(m + 1) * CH],
                            scalar=tcols[m], in1=mrl[:, m * CH:(m + 1) * CH],
                            op0=mybir.AluOpType.is_ge, op1=mybir.AluOpType.mult)
                        # XBAR transpose on the DMA engines: PE/ACT stay free
                        nc.sync.dma_start(eT3[:, q * 4:(q + 1) * 4,
                                              m * 128:(m + 1) * 128],
                                          mrb, transpose=True)

                def decode_ks(q4, psd, k4s):
                    dsl = slice(q4 * 512, (q4 + 1) * 512)
                    for k4 in k4s:
                        rows = slice(k4 * KD * 128, (k4 + 1) * KD * 128)
                        wd = st.tile([128, KD * 512], mybir.dt.bfloat16,
                                     tag="wd", bufs=4, name=f"wd_{q4}_{k4}")
                        nc.sync.dma_start(
                            wd.rearrange("p (k n) -> p k n", k=KD),
                            wd_d[rows, dsl].rearrange("(k p) n -> p k n", p=128))
                        wd3 = wd.rearrange("p (k n) -> p k n", k=KD)
                        for kk in range(KD):
                            k = k4 * KD + kk
                            for m in range(MT):
                                nc.tensor.matmul(
                                    psd[m], eT3[:, k, m * 128:(m + 1) * 128],
                                    wd3[:, kk, :],
                                    start=(k == 0), stop=(k == KTD - 1))

                def evac(q4, psd):
                    dsl = slice(q4 * 512, (q4 + 1) * 512)
                    xh = st.tile([128, MT * 512], mybir.dt.float32,
                                 tag="xh", bufs=1, name=f"xh_{q4}")
                    for m in range(MT):
                        xsl = xh[:, m * 512:(m + 1) * 512]
                        if with_bdec:
                            nc.vector.tensor_add(xsl, psd[m], bd_bc[:, dsl])
                        else:
                            nc.vector.tensor_copy(xsl, psd[m])
                    nc.gpsimd.dma_start(
                        out_d[:, dsl].rearrange("(m p) d -> p m d", p=128),
                        xh.rearrange("p (m d) -> p m d", m=MT))

                # quarter 0 decode chases mask chunk production (k = 4q)
                psd = [psp3.tile([128, 512], mybir.dt.float32, tag="psd",
                                 name=f"psd_0_{m}") for m in range(MT)]
                for q in range(NCH):
                    mask_chunk(q)
                    decode_ks(0, psd, [q])
                evac(0, psd)
                for q4 in range(1, NQ):
                    psd = [psp3.tile([128, 512], mybir.dt.float32, tag="psd",
                                     name=f"psd_{q4}_{m}") for m in range(MT)]
                    decode_ks(q4, psd, range(KTD // KD))
                    evac(q4, psd)

    nc.compile()
    _cache[key] = nc
    return nc


def kernel(x, W_enc, b_enc, W_dec, b_dec):
    import os
    x = np.ascontiguousarray(np.asarray(x, dtype=np.float32))
    W_enc = np.ascontiguousarray(np.asarray(W_enc, dtype=np.float32))
    b_enc = np.asarray(b_enc, dtype=np.float32).reshape(1, DSAE)
    W_dec_bf = np.asarray(W_dec, dtype=np.float32).astype(ml_dtypes.bfloat16)
    b_dec = np.asarray(b_dec, dtype=np.float32).reshape(1, DIN)

    with_bdec = bool(np.any(b_dec))
    if with_bdec:
        x = x - b_dec  # exact on host; encode then needs no subtraction
    xr = round_fp32r(x)
    wr = round_fp32r(W_enc)

    nc = _build(bool(np.any(b_enc)), with_bdec)
    in_maps = []
    for c in range(NCORES):
        in_maps.append({
            "xT": np.ascontiguousarray(xr[c * TPC:(c + 1) * TPC].T),
            "w_enc": wr,
            "b_enc": b_enc,
            "w_dec": W_dec_bf,
            "b_dec": b_dec,
        })
    trace = bool(int(os.environ.get("KERNEL_TRACE", "0")))
    res = run_bass_kernel_spmd(nc, in_maps, core_ids=list(range(NCORES)), trace=trace)
    kernel.last_results = res
    out = np.concatenate([r["xhat"] for r in res.results], axis=0)
    return out.astype(np.float32)
